# revision 24
# baseline (speedup 1.0000x reference)
"""Trainium2 Bass kernel for a 12-layer BERT-style transformer encoder stack.

Reference computation (per layer):
    q,k,v = x@Wq+bq, x@Wk+bk, x@Wv+bv          (x: [S,B,H])
    attn  = softmax(q@k^T / sqrt(HD)) @ v       (per (batch, head))
    x     = LayerNorm(attn@Wo + bo + x) * gamma + beta

Sharding (8 cores): 2-way batch data-parallel x 4-way head tensor-parallel
(Megatron).  Core c handles batch c//4 and heads [4*(c%4), 4*(c%4)+4).
Wq/Wk/Wv are column-sliced, Wo row-sliced; the per-layer partial outputs
(ctx @ Wo_slice) are AllReduce'd within each 4-core quad, chunked by
sequence quarters so communication overlaps attention compute.

On-chip layout: everything lives feature-major ("transposed", [H, S]) so
that the PE contraction dim (partitions) is always the feature dim and no
on-chip transposes are ever needed.

Performance structure (v4):
  - All projections and the probs@V context matmul run in fp8
    (e4m3 weights/activations, e5m2 probs/V) using DoubleRowSwInterleave
    matmuls that contract two 128-row K-tiles per instruction.
  - Scores are computed per HEAD PAIR with PE row tiling: head 2m's K=64
    matmul occupies array rows 0-63 (tile_position (0,0)) and head 2m+1's
    occupies rows 64-127 ((64,0)); the two matmuls execute concurrently in
    independent 32-row groups, so the score phase runs at ~2 cols/cycle
    with no zero-padding of K.  k for a head pair lives in one [128,...]
    tile (kp2), halving k-drain DVE ops and removing all kpad memsets.
  - Both heads' scores for one key t-chunk land in one 2-bank PSUM tile;
    a single ACTIVATE exps them together (N=1024) into a shared per-t-pair
    probs tile [128, 4(head,t), QW] that directly feeds both heads' DRSW
    ctx matmuls.
  - The softmax denominators of a head pair are inverted with ONE
    reciprocal_approx_fast on a [2, QW] row pair.
  - LayerNorm: residual add + square are emitted as single batched DVE ops
    over all 8 feature chunks ([128, 8, QW]); x^T and out^T are single
    [128, HC, S] tiles.  bo is folded into the residual stream host-side
    (xT carries x + bo_eff of the NEXT layer), making the residual a pure
    tensor_tensor add.
  - v8 score/ones constant regions live in two persistent ping-pong tiles,
    memset once instead of per layer.
  - Scalar (ACT) engine runs almost exclusively Exp (one activation table
    for exp+ln); drains and bias adds live on DVE; partition broadcasts on
    GPSIMD.  Note: the chip runs an activity/power throttle (util limit
    0.5-0.8 most of the time), so total engine work - not overlap - is
    what bounds runtime.
"""

import sys

sys.path.insert(0, "/opt/trn_rl_repo")

import numpy as np
import ml_dtypes

import concourse.bass as bass
import concourse.tile as tile
from concourse import bacc
from concourse import mybir
from concourse.bass_utils import run_bass_kernel_spmd

# Route every exp/ln activation to the one table that serves both
# (natural_log_exp_and_others) so the kernel never reloads ACT tables.
import concourse.hw_specs as _hw_specs

if not getattr(_hw_specs, "_one_table_patch", False):
    _orig_get_tables = _hw_specs.get_activation_tables

    def _one_table(arch):
        tabs = _orig_get_tables(arch)
        out = {}
        for name, funcs in tabs.items():
            if name != "natural_log_exp_and_others":
                funcs = {
                    f for f in funcs
                    if f not in (mybir.ActivationFunctionType.Exp,
                                 mybir.ActivationFunctionType.Ln)
                }
            out[name] = funcs
        return out

    _hw_specs.get_activation_tables = _one_table
    _hw_specs._one_table_patch = True
    bacc.get_activation_tables = _one_table

# Problem constants
S, B, H, NH, L = 2048, 2, 1024, 16, 12
HD = H // NH          # 64
EPS = 1e-12
N_CORES = 8
NHL = 4               # heads per core (4-way head split)
DQ = NHL * HD         # 256 local feature cols for q/k/v
HC = H // 128         # 8 h-chunks of 128 partitions
MQ = DQ // 128        # 2 local m-chunks (head pairs)

F16 = mybir.dt.float16
F32 = mybir.dt.float32
F8E4 = mybir.dt.float8e4   # ml_dtypes.float8_e4m3 (max 240)
F8E5 = mybir.dt.float8e5   # ml_dtypes.float8_e5m2

SW = 16.0              # host-side weight pre-scale before e4m3 quantization
DRSW = mybir.MatmulPerfMode.DoubleRowSwInterleave

REPLICA_GROUPS = [[0, 1, 2, 3], [4, 5, 6, 7]]


def build_bass(s=S, l_layers=L, quads=REPLICA_GROUPS, debug=False):
    """Builds the SPMD Bass program (identical on all 8 cores)."""
    QW = s // 4            # sequence quarter width (AR chunk) <= 512
    NT = s // 128          # 128-row t-chunks of the sequence
    NTP = NT // 2          # t-chunk pairs (one fp8 DoubleRow ctx matmul each)
    LAG = 3                # ctx matmul trails exp by LAG t-chunk-pairs

    nc = bacc.Bacc("TRN2", num_devices=N_CORES)

    # ---- I/O ----
    xT0 = nc.dram_tensor("xT0", [HC, 128, s], F16, kind="ExternalInput")
    xT80 = nc.dram_tensor("xT80", [HC, 128, s], F8E4, kind="ExternalInput")
    # wq/wk: canonical SwInterleave layout over c-chunk pairs:
    #   [.., c2, m, 2*128] with stored cols [A(127) B(127) ... A(0) B(0)]
    wq_d = nc.dram_tensor("wq", [l_layers, 128, HC // 2, MQ, 256], F8E4,
                          kind="ExternalInput")
    wk_d = nc.dram_tensor("wk", [l_layers, 128, HC // 2, MQ, 256], F8E4,
                          kind="ExternalInput")
    wv_d = nc.dram_tensor("wv", [l_layers, 128, HC, DQ], F8E4, kind="ExternalInput")
    # wo: FULL Wo per core (AllGather-ctx scheme), one slice per quad member
    # g, rows permuted to match that member's on-chip ctxT8 layout, then
    # canonical SwInterleave over the two m-chunks: [.., g, c, 2*128]
    wo_d = nc.dram_tensor("wo", [l_layers, 128, 4, HC, 256], F8E4,
                          kind="ExternalInput")
    bqk_d = nc.dram_tensor("bqk", [l_layers, 128, 2 * MQ], F32, kind="ExternalInput")
    lnw_d = nc.dram_tensor("lnw", [l_layers, 128, HC, 3], F32, kind="ExternalInput")
    outx = nc.dram_tensor("outx", [HC, 128, s], F32, kind="ExternalOutput")

    from contextlib import ExitStack

    with tile.TileContext(nc) as tc:
        with ExitStack() as ctx:
            pool = lambda *a, **kw: ctx.enter_context(tc.tile_pool(*a, **kw))
            consts = pool(name="consts", bufs=1)
            xTp = pool(name="xT", bufs=1)
            x8p = pool(name="xT8", bufs=1)
            w3p = pool(name="w3", bufs=4)
            wvp = pool(name="wv", bufs=2)
            wop = pool(name="wo", bufs=2)
            smallp = pool(name="small", bufs=2)
            qkp = pool(name="qT8", bufs=4)
            kpp = pool(name="kp2", bufs=4)
            c8p = pool(name="ctxT8", bufs=2)
            cgp = pool(name="ctxg", bufs=2)
            vp = pool(name="vsb", bufs=2)
            prp = pool(name="probs", bufs=5)
            otp = pool(name="outT", bufs=1)
            sqp = pool(name="sq", bufs=2)
            ltp = pool(name="lntmp", bufs=2)
            lrp = pool(name="lnrow", bufs=6)
            rrp = pool(name="rrow", bufs=3)
            fop = pool(name="fout", bufs=2)
            pa = pool(name="pa", bufs=2, space="PSUM")
            pb = pool(name="pb", bufs=2, space="PSUM")
            ps2 = pool(name="ps2", bufs=2, space="PSUM")
            dramp = pool(name="dram", bufs=16, space="DRAM")
            ones16 = consts.tile([128, 128], F16, tag="ones16")
            nc.vector.memset(ones16[:], 1.0)
            eps_sb = consts.tile([128, 1], F32, tag="eps")
            nc.vector.memset(eps_sb[:], EPS)

            # Persistent x^T state: fp16 master (residual stream, carries
            # +bo_eff of the upcoming layer) and an fp8e4m3 shadow used as
            # matmul input.  Both feature-chunk-major single tiles.
            xT = xTp.tile([128, HC, s], F16, tag="xT", name="xT")
            for c in range(HC):
                nc.sync.dma_start(xT[:, c, :], xT0[c, :, :])
            xT8 = x8p.tile([128, HC, s], F8E4, tag="xT8", name="xT8")
            for c in range(HC):
                nc.sync.dma_start(xT8[:, c, :], xT80[c, :, :])

            # v in naturally-interleaved t-pair layout: v8[p, tp, h, slot,
            # par] holds v[t=2*tp+par][p, head h]; 128 slots (ldweights
            # dual-fp8 wants AP elems == 2*128): slots 0..62 zero-pad, slot
            # 63 ones, slot 64+d = dim d.  As the SwInterleave stationary of
            # the ctx matmul (out row r = slot 127-r) this puts the softmax
            # denominator in pctx row 64 and ctx dim d at row 63-d; the
            # reversal is absorbed by the host-side Wo row permutation.
            # Two persistent ping-pong tiles; const regions memset ONCE.
            v8_pp = []
            for i in range(2):
                t = vp.tile([128, NTP, NHL, 128, 2], F8E5, tag="vsb",
                            name=f"v8_{i}")
                nc.gpsimd.memset(t[:, :, :, 0:63, :], 0.0)
                nc.gpsimd.memset(t[:, :, :, 63, :], 1.0)
                v8_pp.append(t)

            def alloc_layer(l):
                st = {"l": l, "last": l == l_layers - 1, "ctxgs": []}
                st["wq"] = w3p.tile([128, HC // 2, MQ, 256], F8E4, tag="w3", name=f"wq{l}")
                st["wk"] = w3p.tile([128, HC // 2, MQ, 256], F8E4, tag="w3", name=f"wk{l}")
                st["wv"] = wvp.tile([128, HC, DQ], F8E4, tag="wv", name=f"wv{l}")
                nc.sync.dma_start(st["wq"][:], wq_d[l, :, :, :, :])
                nc.sync.dma_start(st["wk"][:], wk_d[l, :, :, :, :])
                nc.sync.dma_start(st["wv"][:], wv_d[l, :, :, :])
                st["wo"] = wop.tile([128, 4, HC, 256], F8E4, tag="wo", name=f"wo{l}")
                nc.sync.dma_start(st["wo"][:], wo_d[l, :, :, :, :])
                st["bqk"] = smallp.tile([128, 2 * MQ], F32, tag="bqk", name=f"bqk{l}")
                nc.sync.dma_start(st["bqk"][:], bqk_d[l, :, :])
                st["lnw"] = smallp.tile([128, HC, 3], F32, tag="lnw", name=f"lnw{l}")
                nc.sync.dma_start(st["lnw"][:], lnw_d[l, :, :, :])
                # q lands in one fp8 tile per m-pair (head 2m dims on
                # partitions 0:64, head 2m+1 on 64:128); k likewise in one
                # kp2 tile per m-pair, keyed by key t-chunk.
                st["qT8"] = [qkp.tile([128, s], F8E4, tag="qT8", name=f"qT{l}_{m}")
                             for m in range(MQ)]
                st["kp2"] = [kpp.tile([128, NT, 128], F8E4, tag="kp2",
                                      name=f"kp{l}_{m}") for m in range(MQ)]
                st["v8"] = v8_pp[l % 2]
                # ctxT8 holds 16*ctx/l in fp8e4m3: [128, m, s] so the Wo
                # DoubleRow matmul can pair the two m-chunks.
                st["ctxT8"] = c8p.tile([128, MQ, s], F8E4, tag="ctxT8",
                                       name=f"ctxT8{l}")
                st["outT"] = otp.tile([128, HC, s], F16, tag="outT",
                                      name=f"outT{l}")
                return st

            def proj_qk(st, qi):
                """q/k projections for quarter qi."""
                l = st["l"]
                sw = slice(qi * QW, (qi + 1) * QW)
                for m in range(MQ):
                    for dst, w_sb, bcol in (("q", st["wq"], m), ("k", st["wk"], MQ + m)):
                        ps = pa.tile([128, QW], F32, tag="pa")
                        for c2 in range(HC // 2):
                            nc.tensor.matmul(
                                ps[:],
                                w_sb[:, c2, m, :],
                                xT8[:, 2 * c2:2 * c2 + 2, sw],
                                start=(c2 == 0),
                                stop=(c2 == HC // 2 - 1),
                                perf_mode=DRSW,
                            )
                        if dst == "q":
                            nc.vector.tensor_scalar_add(
                                st["qT8"][m][:, sw], ps[:],
                                st["bqk"][:, bcol:bcol + 1]
                            )
                        else:
                            nc.vector.tensor_scalar_add(
                                st["kp2"][m][:, 4 * qi:4 * qi + 4, :]
                                .rearrange("p t n -> p (t n)"),
                                ps[:],
                                st["bqk"][:, bcol:bcol + 1],
                            )

            def proj_v(st, qi):
                """v projections for t-chunks 4qi..4qi+3; two t-chunks share
                one PSUM bank and a single strided drain."""
                for th in range(2):
                    t0 = 4 * qi + 2 * th
                    tpi = 2 * qi + th
                    ps = pa.tile([128, 2, DQ], F32, tag="pa")
                    for half in range(2):
                        t = t0 + half
                        for c in range(HC):
                            nc.tensor.matmul(
                                ps[:, half, :],
                                xT8[:, c, t * 128:(t + 1) * 128],
                                st["wv"][:, c, :],
                                start=(c == 0),
                                stop=(c == HC - 1),
                            )
                    nc.vector.tensor_copy(
                        out=st["v8"][:, tpi, :, 64:128, :],
                        in_=ps[:].rearrange("p two (h d) -> p h d two", h=NHL),
                    )

            def emit_ctx_ag(st, qj):
                # AllGather the (tiny, fp8) normalized ctx for quarter qj
                # across the quad; each core then runs the FULL Wo matmul
                # locally.  ~8x less CC traffic than AllReducing fp16 deltas.
                l = st["l"]
                swj = slice(qj * QW, (qj + 1) * QW)
                agin = dramp.tile([128, MQ, QW], F8E4, tag="agin",
                                  name=f"agin{l}_{qj}")
                agout = dramp.tile([4, 128, MQ, QW], F8E4, tag="agout",
                                   name=f"agout{l}_{qj}")
                nc.sync.dma_start(agin[:], st["ctxT8"][:, :, swj])
                nc.gpsimd.collective_compute(
                    "AllGather",
                    mybir.AluOpType.bypass,
                    replica_groups=quads,
                    ins=[agin[:].opt()],
                    outs=[agout[:].opt()],
                )
                ctxg = cgp.tile([128, 4, MQ, QW], F8E4, tag="ctxg",
                                name=f"ctxg{l}_{qj}")
                for g in range(4):
                    nc.sync.dma_start(ctxg[:, g, :, :], agout[g, :, :, :])
                st["ctxgs"].append(ctxg)

            def attn_pair(st, qi, m, slots=None):
                """Scores/exp/ctx/normalize for head pair (2m, 2m+1) over
                query quarter qi.  The two heads' K=64 score matmuls run
                concurrently in PE row groups 0-63 / 64-127.  `slots` maps
                tp -> closures emitted at that tp's loop head, used to
                interleave proj / LN PE work into the exp-bound phase."""
                l = st["l"]
                sw = slice(qi * QW, (qi + 1) * QW)
                kp = st["kp2"][m]
                q = st["qT8"][m]
                pctx = [pb.tile([128, QW], F32, tag="pb",
                                name=f"pc{l}_{qi}_{m}_{h}") for h in range(2)]
                probs = [None] * NTP

                def ctx_mm(tp):
                    for h in range(2):
                        nc.tensor.matmul(
                            pctx[h][:],
                            st["v8"][:, tp, 2 * m + h, :, :]
                            .rearrange("p d two -> p (d two)"),
                            probs[tp][:, 2 * h:2 * h + 2, :],
                            start=(tp == 0),
                            stop=(tp == NTP - 1),
                            perf_mode=DRSW,
                        )

                for tp in range(NTP):
                    if slots:
                        for fn in slots.get(tp, []):
                            fn()
                    probs[tp] = prp.tile([128, 4, QW], F8E5, tag="probs",
                                         name=f"pr{l}_{qi}_{m}_{tp}")
                    # probs slots: (h-even t-even, h-even t-odd,
                    #               h-odd t-even, h-odd t-odd)
                    pview = probs[tp][:].rearrange(
                        "p (two pair) n -> p two pair n", two=2)
                    for half in range(2):
                        t = 2 * tp + half
                        ss = ps2.tile([128, 2, QW], F32, tag="ps2")
                        nc.tensor.matmul(ss[:, 0, :], kp[0:64, t, :],
                                         q[0:64, sw], start=True, stop=True)
                        nc.tensor.matmul(ss[:, 1, :], kp[64:128, t, :],
                                         q[64:128, sw], start=True, stop=True)
                        nc.scalar.activation(
                            out=pview[:, :, half, :],
                            in_=ss[:],
                            func=mybir.ActivationFunctionType.Exp,
                            scale=float(1.0 / (np.sqrt(HD) * SW * SW)),
                        )
                    if tp >= LAG:
                        ctx_mm(tp - LAG)
                for tp in range(NTP - LAG, NTP):
                    ctx_mm(tp)

                # normalize both heads: ctx^T * (16 / l[s']), l at psum row
                # 64, ctx dim d at psum row 63-d (SwInterleave reversal).
                # reciprocal_approx_fast misreads PSUM inputs on hw: stage
                # the denominator rows to SBUF first.  (DVE partition bases
                # must be 32-aligned, so rows stay in their own tiles.)
                for h in range(2):
                    lrow = rrp.tile([1, QW], F32, tag="lrow",
                                    name=f"lr_{l}_{qi}_{m}_{h}")
                    nc.vector.tensor_copy(out=lrow[:], in_=pctx[h][64:65, :])
                    r32 = rrp.tile([1, QW], F32, tag="rrow",
                                   name=f"r32_{l}_{qi}_{m}_{h}")
                    nc.vector.reciprocal_approx_fast(out=r32[:], in_=lrow[:])
                    bcs = rrp.tile([64, QW], F32, tag="bcs",
                                   name=f"bcs{l}_{qi}_{m}_{h}")
                    nc.gpsimd.partition_broadcast(bcs[:], r32[:])
                    nc.vector.tensor_mul(
                        out=st["ctxT8"][64 * h:64 * h + 64, m, sw],
                        in0=pctx[h][0:64, :],
                        in1=bcs[:],
                    )

            def ln_wo(st, qi, c0, c1):
                """Full Wo matmul over the gathered quad ctx for feature
                chunks [c0, c1), then out^T = delta/(SW*SW) + (x^T + bo_eff)
                [bo folded host-side] and the square for the LN stats."""
                l = st["l"]
                outT = st["outT"]
                ctxg = st["ctxgs"][qi]
                sw = slice(qi * QW, (qi + 1) * QW)
                if c0 == 0:
                    st.setdefault("sqtq", {})[qi] = sqp.tile(
                        [128, HC, QW], F16, tag="sq", bufs=1,
                        name=f"sqt{l}_{qi}")
                sqt = st["sqtq"][qi]
                for c in range(c0, c1):
                    pd = pa.tile([128, QW], F32, tag="pa", name=f"pd{l}_{qi}_{c}")
                    for g in range(4):
                        nc.tensor.matmul(
                            pd[:],
                            st["wo"][:, g, c, :],
                            ctxg[:, g, :, :],
                            start=(g == 0),
                            stop=(g == 3),
                            perf_mode=DRSW,
                        )
                    # psum holds 256*delta (16 from ctx scale, 16 from Wo)
                    nc.vector.scalar_tensor_tensor(
                        out=outT[:, c, sw],
                        in0=pd[:],
                        scalar=1.0 / (SW * SW),
                        in1=xT[:, c, sw],
                        op0=mybir.AluOpType.mult,
                        op1=mybir.AluOpType.add,
                    )
                    nc.vector.tensor_mul(
                        out=sqt[:, c, :], in0=outT[:, c, sw], in1=outT[:, c, sw]
                    )

            def ln_fin(st, qi):
                l, last = st["l"], st["last"]
                outT, lnw_sb = st["outT"], st["lnw"]
                sqt = st["sqtq"].pop(qi)
                sw = slice(qi * QW, (qi + 1) * QW)
                # pa, not pb: ln_fin can sit inside a head pair whose two
                # live pctx accumulators own the whole pb ring (deadlock).
                pst = pa.tile([128, QW], F32, tag="pa")
                for c in range(HC):
                    nc.tensor.matmul(
                        pst[0:1, :], ones16[:, 0:1], outT[:, c, sw],
                        start=(c == 0), stop=(c == HC - 1),
                        skip_group_check=True,
                    )
                    nc.tensor.matmul(
                        pst[32:33, :], ones16[:, 0:1], sqt[:, c, :],
                        start=(c == 0), stop=(c == HC - 1),
                        skip_group_check=True,
                    )
                # LN finalize for this quarter (read sums straight from
                # PSUM: mean = sum/H, var = sumsq/H - mean^2)
                m_sb = lrp.tile([1, QW], F16, tag="lnrow", name=f"m{l}_{qi}")
                nc.vector.tensor_scalar_mul(m_sb[:], pst[0:1, :], 1.0 / H)
                m2 = lrp.tile([1, QW], F16, tag="lnrow", name=f"m2{l}_{qi}")
                nc.vector.tensor_mul(m2[:], m_sb[:], m_sb[:])
                var = lrp.tile([1, QW], F16, tag="lnrow", name=f"va{l}_{qi}")
                nc.vector.scalar_tensor_tensor(
                    out=var[:], in0=pst[32:33, :], scalar=1.0 / H, in1=m2[:],
                    op0=mybir.AluOpType.mult, op1=mybir.AluOpType.subtract,
                )
                # rstd = exp(-0.5 * ln(var + eps)); ln+exp share one ACT table
                lnv = lrp.tile([1, QW], F16, tag="lnrow", name=f"lv{l}_{qi}")
                nc.scalar.activation(
                    out=lnv[:], in_=var[:],
                    func=mybir.ActivationFunctionType.Ln,
                    bias=eps_sb[0:1, :],
                )
                rstd = lrp.tile([1, QW], F16, tag="lnrow", name=f"rs{l}_{qi}")
                nc.scalar.activation(
                    out=rstd[:], in_=lnv[:],
                    func=mybir.ActivationFunctionType.Exp,
                    scale=-0.5,
                )
                # broadcast stats across partitions (gpsimd, off the PE)
                mbs = ltp.tile([128, QW], F16, tag="lntmp", name=f"mbs{l}_{qi}")
                nc.gpsimd.partition_broadcast(mbs[:], m_sb[:])
                rbs = ltp.tile([128, QW], F16, tag="lntmp", name=f"rbs{l}_{qi}")
                nc.gpsimd.partition_broadcast(rbs[:], rstd[:])
                for c in range(HC):
                    tmp = sqp.tile([128, QW], F16, tag="lt", name=f"lt{l}_{qi}_{c}")
                    nc.vector.tensor_sub(out=tmp[:], in0=outT[:, c, sw], in1=mbs[:])
                    nc.vector.scalar_tensor_tensor(
                        out=tmp[:], in0=tmp[:],
                        scalar=lnw_sb[:, c, 0:1], in1=rbs[:],
                        op0=mybir.AluOpType.mult, op1=mybir.AluOpType.mult,
                    )
                    if last:
                        fo = fop.tile([128, QW], F32, tag="fout")
                        nc.vector.tensor_scalar_add(
                            fo[:], tmp[:], lnw_sb[:, c, 1:2]
                        )
                        nc.sync.dma_start(outx[c, :, sw], fo[:])
                    else:
                        nc.vector.tensor_scalar_add(
                            xT8[:, c, sw], tmp[:], lnw_sb[:, c, 1:2]
                        )
                        # xT gets beta + bo_eff of the NEXT layer (slot 2)
                        nc.vector.tensor_scalar_add(
                            xT[:, c, sw], tmp[:], lnw_sb[:, c, 2:3]
                        )

            # ---- schedule: attention's exp chain is the wall; all other PE
            # work (projections of later quarters, the previous quarter's
            # Wo+LN) is sliced into ~1-3us items and slotted between the
            # exp-bound t-pairs so the PE never serializes ahead of the ACT
            # engine.  Quarter qi's scores need proj(qk) of ALL quarters
            # (keys span the sequence): key-quarter X is consumed from tp=2X
            # on, so proj(qX) sits at slot 2X of the very first head pair.
            prev = None
            for l in range(l_layers):
                st = alloc_layer(l)
                proj_qk(st, 0)
                proj_v(st, 0)
                for qi in range(4):
                    if qi == 0:
                        if prev is not None:
                            # ln(l-1, q3): its AG launched at the end of the
                            # previous layer, ~15us ago -- safe from tp1 on.
                            pv = prev
                            s0 = {
                                0: [lambda: proj_qk(st, 1)],
                                1: [lambda: ln_wo(pv, 3, 0, 4)],
                                2: [lambda: ln_wo(pv, 3, 4, 8)],
                                3: [lambda: ln_fin(pv, 3),
                                    lambda: proj_v(st, 1)],
                                4: [lambda: proj_qk(st, 2)],
                                5: [lambda: proj_v(st, 2)],
                                6: [lambda: proj_qk(st, 3)],
                                7: [lambda: proj_v(st, 3)],
                            }
                        else:
                            s0 = {
                                0: [lambda: proj_qk(st, 1)],
                                1: [lambda: proj_v(st, 1)],
                                2: [lambda: proj_qk(st, 2)],
                                3: [lambda: proj_v(st, 2)],
                                4: [lambda: proj_qk(st, 3)],
                                5: [lambda: proj_v(st, 3)],
                            }
                        s1 = {}
                    else:
                        # ln(qi-1) PE work rides in PAIR1 of quarter qi: by
                        # then its AllGather has had all of pair0 (~18us) to
                        # complete, so the Wo matmuls never head-of-line
                        # block the scores.
                        s0 = {}
                        s1 = {tp: [lambda st=st, q=qi - 1, c=2 * tp - 4:
                                   ln_wo(st, q, c, c + 2)]
                              for tp in range(2, 6)}
                    attn_pair(st, qi, 0, slots=s0)
                    attn_pair(st, qi, 1, slots=s1)
                    if qi >= 1:
                        ln_fin(st, qi - 1)
                    # AG has no PE work -> emit immediately, no head-of-line
                    # blocking of the next quarter's scores
                    emit_ctx_ag(st, qi)
                prev = st
            ln_wo(prev, 3, 0, 8)
            ln_fin(prev, 3)
    nc.compile()
    return nc


def make_in_maps(inputs, s=S, l_layers=L):
    """Host-side sharding: returns one input dict per core."""
    x = np.asarray(inputs["input_tensor"], dtype=np.float32)      # [s, B, H]
    Wq = np.asarray(inputs["Wq"], dtype=np.float32)[:l_layers]
    Wk = np.asarray(inputs["Wk"], dtype=np.float32)[:l_layers]
    Wv = np.asarray(inputs["Wv"], dtype=np.float32)[:l_layers]
    Wo = np.asarray(inputs["Wo"], dtype=np.float32)[:l_layers]
    bq = np.asarray(inputs["bq"], dtype=np.float32)[:l_layers]
    bk = np.asarray(inputs["bk"], dtype=np.float32)[:l_layers]
    bv = np.asarray(inputs["bv"], dtype=np.float32)[:l_layers]
    bo = np.asarray(inputs["bo"], dtype=np.float32)[:l_layers]
    gamma = np.asarray(inputs["gamma"], dtype=np.float32)[:l_layers]
    beta = np.asarray(inputs["beta"], dtype=np.float32)[:l_layers]
    ll = l_layers

    # bv passes through the softmax-weighted sum exactly: fold bv@Wo into bo.
    bo_eff = bo + np.einsum("lh,lhk->lk", bv, Wo)
    # xT carries x + bo_eff of the upcoming layer; lnw slot 2 = beta +
    # bo_eff(next layer) so the residual add is a pure tensor_tensor op.
    sl2 = beta.copy()
    sl2[:ll - 1] += bo_eff[1:]

    def chunkP(a, n_out):
        # [..., n_out*128, inner] -> [..., 128, n_out, inner] feature-chunked
        sh = a.shape
        a = a.reshape(*sh[:-2], n_out, 128, sh[-1])
        return np.moveaxis(a, -3, -2)  # -> [..., 128, n_out, inner]

    e4 = ml_dtypes.float8_e4m3

    def sw_interleave(A, Bm):
        # A, Bm: [..., K, M] -> [..., K, 2M] canonical SwInterleave layout:
        # stored cols [A(M-1) B(M-1) ... A(0) B(0)]
        st = np.stack([A[..., ::-1], Bm[..., ::-1]], axis=-1)
        return st.reshape(*st.shape[:-2], -1)

    def qk_prep(W):
        # [L,H,DQ]*SW -> [L, 128, HC//2, MQ, 256] SwInterleave over c-pairs
        Wc = (W * SW).reshape(ll, HC, 128, DQ)       # [L, c, p, DQ]
        out = np.empty((ll, 128, HC // 2, MQ, 256), np.float32)
        for c2 in range(HC // 2):
            for m in range(MQ):
                A = Wc[:, 2 * c2, :, m * 128:(m + 1) * 128]
                Bm = Wc[:, 2 * c2 + 1, :, m * 128:(m + 1) * 128]
                out[:, :, c2, m, :] = sw_interleave(A, Bm)
        return out

    # ctxT8 partition p (within m-chunk par) holds head 2*par + (p>=64),
    # dim d = 63 - (p % 64); permute Wo rows to match before interleaving.
    k_idx = np.arange(128)

    def wo_prep(Wc):
        # Wc: [L, DQ, H]*SW -> [L, 128, HC, 256] (rows permuted + interleaved)
        Wp = np.empty((ll, 2, 128, H), np.float32)
        for par in range(2):
            f = 64 * (2 * par + (k_idx >= 64)) + (63 - (k_idx % 64))
            Wp[:, par, :, :] = Wc[:, f, :] * SW
        out = np.empty((ll, 128, HC, 256), np.float32)
        for c in range(HC):
            out[:, :, c, :] = sw_interleave(
                Wp[:, 0, :, c * 128:(c + 1) * 128],
                Wp[:, 1, :, c * 128:(c + 1) * 128],
            )
        return out

    # Full Wo per core (AllGather-ctx): one permuted+interleaved slice per
    # quad member g; identical on every core.
    wo = np.ascontiguousarray(np.stack(
        [wo_prep(Wo[:, DQ * g:DQ * (g + 1), :]) for g in range(4)], axis=2
    ))

    in_maps = []
    for core in range(N_CORES):
        g, j = core // 4, core % 4
        cols = slice(DQ * j, DQ * (j + 1))
        xT = np.ascontiguousarray(x[:, g, :].T).reshape(HC, 128, s)
        xTb = xT + bo_eff[0].reshape(HC, 128, 1)
        wq = np.ascontiguousarray(qk_prep(Wq[:, :, cols]))
        wk = np.ascontiguousarray(qk_prep(Wk[:, :, cols]))
        wv = np.ascontiguousarray(chunkP(Wv[:, :, cols] * SW, HC))
        bqs = bq[:, cols].reshape(ll, MQ, 128).transpose(0, 2, 1)  # [L,128,MQ]
        bks = bk[:, cols].reshape(ll, MQ, 128).transpose(0, 2, 1)
        bqk = np.ascontiguousarray(np.concatenate([bqs, bks], axis=2)) * SW
        lnw = np.stack(
            [
                gamma.reshape(ll, HC, 128).transpose(0, 2, 1),
                beta.reshape(ll, HC, 128).transpose(0, 2, 1),
                sl2.reshape(ll, HC, 128).transpose(0, 2, 1),
            ],
            axis=3,
        )                                                          # [L,128,HC,3]
        in_maps.append(
            {
                "xT0": xTb.astype(np.float16),
                "xT80": xT.astype(e4),
                "wq": wq.astype(e4),
                "wk": wk.astype(e4),
                "wv": wv.astype(e4),
                "wo": wo.astype(e4),
                "bqk": bqk.astype(np.float32),
                "lnw": np.ascontiguousarray(lnw).astype(np.float32),
            }
        )
    return in_maps


_NC_CACHE = {}


def kernel(**inputs) -> np.ndarray:
    in_maps = make_in_maps(inputs)
    key = (S, L)
    if key not in _NC_CACHE:
        _NC_CACHE[key] = build_bass()
    nc = _NC_CACHE[key]
    res = run_bass_kernel_spmd(nc, in_maps, core_ids=list(range(N_CORES)))
    out = np.empty((S, B, H), dtype=np.float32)
    for g, core in ((0, 0), (1, 4)):
        xt = res.results[core]["outx"].reshape(H, S)
        out[:, g, :] = xt.T
    return out


# revision 27
# speedup vs baseline: 1.1221x; 1.1221x over previous
"""Trainium2 Bass kernel for a 12-layer BERT-style transformer encoder stack.

Reference computation (per layer):
    q,k,v = x@Wq+bq, x@Wk+bk, x@Wv+bv          (x: [S,B,H])
    attn  = softmax(q@k^T / sqrt(HD)) @ v       (per (batch, head))
    x     = LayerNorm(attn@Wo + bo + x) * gamma + beta

Sharding (8 cores): 2-way batch data-parallel x 4-way head tensor-parallel
(Megatron).  Core c handles batch c//4 and heads [4*(c%4), 4*(c%4)+4).
Wq/Wk/Wv are column-sliced, Wo row-sliced; the per-layer partial outputs
(ctx @ Wo_slice) are AllReduce'd within each 4-core quad, chunked by
sequence quarters so communication overlaps attention compute.

On-chip layout: everything lives feature-major ("transposed", [H, S]) so
that the PE contraction dim (partitions) is always the feature dim and no
on-chip transposes are ever needed.

Performance structure (v4):
  - All projections and the probs@V context matmul run in fp8
    (e4m3 weights/activations, e5m2 probs/V) using DoubleRowSwInterleave
    matmuls that contract two 128-row K-tiles per instruction.
  - Scores are computed per HEAD PAIR with PE row tiling: head 2m's K=64
    matmul occupies array rows 0-63 (tile_position (0,0)) and head 2m+1's
    occupies rows 64-127 ((64,0)); the two matmuls execute concurrently in
    independent 32-row groups, so the score phase runs at ~2 cols/cycle
    with no zero-padding of K.  k for a head pair lives in one [128,...]
    tile (kp2), halving k-drain DVE ops and removing all kpad memsets.
  - Both heads' scores for one key t-chunk land in one 2-bank PSUM tile;
    a single ACTIVATE exps them together (N=1024) into a shared per-t-pair
    probs tile [128, 4(head,t), QW] that directly feeds both heads' DRSW
    ctx matmuls.
  - The softmax denominators of a head pair are inverted with ONE
    reciprocal_approx_fast on a [2, QW] row pair.
  - LayerNorm: residual add + square are emitted as single batched DVE ops
    over all 8 feature chunks ([128, 8, QW]); x^T and out^T are single
    [128, HC, S] tiles.  bo is folded into the residual stream host-side
    (xT carries x + bo_eff of the NEXT layer), making the residual a pure
    tensor_tensor add.
  - v8 score/ones constant regions live in two persistent ping-pong tiles,
    memset once instead of per layer.
  - Scalar (ACT) engine runs almost exclusively Exp (one activation table
    for exp+ln); drains and bias adds live on DVE; partition broadcasts on
    GPSIMD.  Note: the chip runs an activity/power throttle (util limit
    0.5-0.8 most of the time), so total engine work - not overlap - is
    what bounds runtime.
"""

import sys

sys.path.insert(0, "/opt/trn_rl_repo")

import numpy as np
import ml_dtypes

import concourse.bass as bass
import concourse.tile as tile
from concourse import bacc
from concourse import mybir
from concourse.bass_utils import run_bass_kernel_spmd

# Route every exp/ln activation to the one table that serves both
# (natural_log_exp_and_others) so the kernel never reloads ACT tables.
import concourse.hw_specs as _hw_specs

if not getattr(_hw_specs, "_one_table_patch", False):
    _orig_get_tables = _hw_specs.get_activation_tables

    def _one_table(arch):
        tabs = _orig_get_tables(arch)
        out = {}
        for name, funcs in tabs.items():
            if name != "natural_log_exp_and_others":
                funcs = {
                    f for f in funcs
                    if f not in (mybir.ActivationFunctionType.Exp,
                                 mybir.ActivationFunctionType.Ln)
                }
            out[name] = funcs
        return out

    _hw_specs.get_activation_tables = _one_table
    _hw_specs._one_table_patch = True
    bacc.get_activation_tables = _one_table

# Problem constants
S, B, H, NH, L = 2048, 2, 1024, 16, 12
HD = H // NH          # 64
EPS = 1e-12
N_CORES = 8
NHL = 4               # heads per core (4-way head split)
DQ = NHL * HD         # 256 local feature cols for q/k/v
HC = H // 128         # 8 h-chunks of 128 partitions
MQ = DQ // 128        # 2 local m-chunks (head pairs)

F16 = mybir.dt.float16
F32 = mybir.dt.float32
F8E4 = mybir.dt.float8e4   # ml_dtypes.float8_e4m3 (max 240)
F8E5 = mybir.dt.float8e5   # ml_dtypes.float8_e5m2

SW = 16.0              # host-side weight pre-scale before e4m3 quantization
DRSW = mybir.MatmulPerfMode.DoubleRowSwInterleave

REPLICA_GROUPS = [[0, 1, 2, 3], [4, 5, 6, 7]]


def build_bass(s=S, l_layers=L, quads=REPLICA_GROUPS, debug=False):
    """Builds the SPMD Bass program (identical on all 8 cores)."""
    QW = s // 4            # sequence quarter width (AR chunk) <= 512
    NT = s // 128          # 128-row t-chunks of the sequence
    NTP = NT // 2          # t-chunk pairs (one fp8 DoubleRow ctx matmul each)
    LAG = 3                # ctx matmul trails exp by LAG t-chunk-pairs

    nc = bacc.Bacc("TRN2", num_devices=N_CORES)

    # ---- I/O ----
    xT0 = nc.dram_tensor("xT0", [HC, 128, s], F16, kind="ExternalInput")
    xT80 = nc.dram_tensor("xT80", [HC, 128, s], F8E4, kind="ExternalInput")
    # wq/wk: canonical SwInterleave layout over c-chunk pairs:
    #   [.., c2, m, 2*128] with stored cols [A(127) B(127) ... A(0) B(0)]
    wq_d = nc.dram_tensor("wq", [l_layers, 128, HC // 2, MQ, 256], F8E4,
                          kind="ExternalInput")
    wk_d = nc.dram_tensor("wk", [l_layers, 128, HC // 2, MQ, 256], F8E4,
                          kind="ExternalInput")
    wv_d = nc.dram_tensor("wv", [l_layers, 128, HC, DQ], F8E4, kind="ExternalInput")
    # wo: FULL Wo per core (AllGather-ctx scheme), one slice per quad member
    # g, rows permuted to match that member's on-chip ctxT8 layout, then
    # canonical SwInterleave over the two m-chunks: [.., g, c, 2*128]
    wo_d = nc.dram_tensor("wo", [l_layers, 128, 4, HC, 256], F8E4,
                          kind="ExternalInput")
    bqk_d = nc.dram_tensor("bqk", [l_layers, 128, 2 * MQ], F32, kind="ExternalInput")
    lnw_d = nc.dram_tensor("lnw", [l_layers, 128, HC, 3], F32, kind="ExternalInput")
    outx = nc.dram_tensor("outx", [HC, 128, s], F32, kind="ExternalOutput")

    from contextlib import ExitStack

    with tile.TileContext(nc) as tc:
        with ExitStack() as ctx:
            pool = lambda *a, **kw: ctx.enter_context(tc.tile_pool(*a, **kw))
            consts = pool(name="consts", bufs=1)
            xTp = pool(name="xT", bufs=1)
            x8p = pool(name="xT8", bufs=1)
            w3p = pool(name="w3", bufs=4)
            wvp = pool(name="wv", bufs=2)
            wop = pool(name="wo", bufs=2)
            smallp = pool(name="small", bufs=2)
            qkp = pool(name="qT8", bufs=4)
            kpp = pool(name="kp2", bufs=4)
            c8p = pool(name="ctxT8", bufs=2)
            cgp = pool(name="ctxg", bufs=2)
            vp = pool(name="vsb", bufs=2)
            prp = pool(name="probs", bufs=5)
            otp = pool(name="outT", bufs=1)
            sqp = pool(name="sq", bufs=2)
            ltp = pool(name="lntmp", bufs=2)
            lrp = pool(name="lnrow", bufs=6)
            rrp = pool(name="rrow", bufs=3)
            fop = pool(name="fout", bufs=2)
            pa = pool(name="pa", bufs=2, space="PSUM")
            pb = pool(name="pb", bufs=2, space="PSUM")
            ps2 = pool(name="ps2", bufs=2, space="PSUM")
            dramp = pool(name="dram", bufs=16, space="DRAM")
            ones16 = consts.tile([128, 128], F16, tag="ones16")
            nc.vector.memset(ones16[:], 1.0)
            eps_sb = consts.tile([128, 1], F32, tag="eps")
            nc.vector.memset(eps_sb[:], EPS)

            # Persistent x^T state: fp16 master (residual stream, carries
            # +bo_eff of the upcoming layer) and an fp8e4m3 shadow used as
            # matmul input.  Both feature-chunk-major single tiles.
            xT = xTp.tile([128, HC, s], F16, tag="xT", name="xT")
            for c in range(HC):
                nc.sync.dma_start(xT[:, c, :], xT0[c, :, :])
            xT8 = x8p.tile([128, HC, s], F8E4, tag="xT8", name="xT8")
            for c in range(HC):
                nc.sync.dma_start(xT8[:, c, :], xT80[c, :, :])

            # v in naturally-interleaved t-pair layout: v8[p, tp, h, slot,
            # par] holds v[t=2*tp+par][p, head h]; 128 slots (ldweights
            # dual-fp8 wants AP elems == 2*128): slots 0..62 zero-pad, slot
            # 63 ones, slot 64+d = dim d.  As the SwInterleave stationary of
            # the ctx matmul (out row r = slot 127-r) this puts the softmax
            # denominator in pctx row 64 and ctx dim d at row 63-d; the
            # reversal is absorbed by the host-side Wo row permutation.
            # Two persistent ping-pong tiles; const regions memset ONCE.
            v8_pp = []
            for i in range(2):
                t = vp.tile([128, NTP, NHL, 128, 2], F8E5, tag="vsb",
                            name=f"v8_{i}")
                nc.gpsimd.memset(t[:, :, :, 0:63, :], 0.0)
                nc.gpsimd.memset(t[:, :, :, 63, :], 1.0)
                v8_pp.append(t)

            def alloc_layer(l):
                st = {"l": l, "last": l == l_layers - 1, "ctxgs": []}
                st["wq"] = w3p.tile([128, HC // 2, MQ, 256], F8E4, tag="w3", name=f"wq{l}")
                st["wk"] = w3p.tile([128, HC // 2, MQ, 256], F8E4, tag="w3", name=f"wk{l}")
                st["wv"] = wvp.tile([128, HC, DQ], F8E4, tag="wv", name=f"wv{l}")
                nc.sync.dma_start(st["wq"][:], wq_d[l, :, :, :, :])
                nc.sync.dma_start(st["wk"][:], wk_d[l, :, :, :, :])
                nc.sync.dma_start(st["wv"][:], wv_d[l, :, :, :])
                st["wo"] = wop.tile([128, 4, HC, 256], F8E4, tag="wo", name=f"wo{l}")
                nc.sync.dma_start(st["wo"][:], wo_d[l, :, :, :, :])
                st["bqk"] = smallp.tile([128, 2 * MQ], F32, tag="bqk", name=f"bqk{l}")
                nc.sync.dma_start(st["bqk"][:], bqk_d[l, :, :])
                st["lnw"] = smallp.tile([128, HC, 3], F32, tag="lnw", name=f"lnw{l}")
                nc.sync.dma_start(st["lnw"][:], lnw_d[l, :, :, :])
                # q lands in one fp8 tile per m-pair (head 2m dims on
                # partitions 0:64, head 2m+1 on 64:128); k likewise in one
                # kp2 tile per m-pair, keyed by key t-chunk.
                st["qT8"] = [qkp.tile([128, s], F8E4, tag="qT8", name=f"qT{l}_{m}")
                             for m in range(MQ)]
                st["kp2"] = [kpp.tile([128, NT, 128], F8E4, tag="kp2",
                                      name=f"kp{l}_{m}") for m in range(MQ)]
                st["v8"] = v8_pp[l % 2]
                # ctxT8 holds 16*ctx/l in fp8e4m3: [128, m, s] so the Wo
                # DoubleRow matmul can pair the two m-chunks.
                st["ctxT8"] = c8p.tile([128, MQ, s], F8E4, tag="ctxT8",
                                       name=f"ctxT8{l}")
                st["outT"] = otp.tile([128, HC, s], F16, tag="outT",
                                      name=f"outT{l}")
                return st

            def proj_qk(st, qi):
                """q/k projections for quarter qi."""
                l = st["l"]
                sw = slice(qi * QW, (qi + 1) * QW)
                for m in range(MQ):
                    for dst, w_sb, bcol in (("q", st["wq"], m), ("k", st["wk"], MQ + m)):
                        ps = pa.tile([128, QW], F32, tag="pa")
                        for c2 in range(HC // 2):
                            nc.tensor.matmul(
                                ps[:],
                                w_sb[:, c2, m, :],
                                xT8[:, 2 * c2:2 * c2 + 2, sw],
                                start=(c2 == 0),
                                stop=(c2 == HC // 2 - 1),
                                perf_mode=DRSW,
                            )
                        if dst == "q":
                            nc.vector.tensor_scalar_add(
                                st["qT8"][m][:, sw], ps[:],
                                st["bqk"][:, bcol:bcol + 1]
                            )
                        else:
                            nc.vector.tensor_scalar_add(
                                st["kp2"][m][:, 4 * qi:4 * qi + 4, :]
                                .rearrange("p t n -> p (t n)"),
                                ps[:],
                                st["bqk"][:, bcol:bcol + 1],
                            )

            def proj_v(st, qi):
                """v projections for t-chunks 4qi..4qi+3; two t-chunks share
                one PSUM bank and a single strided drain."""
                for th in range(2):
                    t0 = 4 * qi + 2 * th
                    tpi = 2 * qi + th
                    ps = pa.tile([128, 2, DQ], F32, tag="pa")
                    for half in range(2):
                        t = t0 + half
                        for c in range(HC):
                            nc.tensor.matmul(
                                ps[:, half, :],
                                xT8[:, c, t * 128:(t + 1) * 128],
                                st["wv"][:, c, :],
                                start=(c == 0),
                                stop=(c == HC - 1),
                            )
                    nc.vector.tensor_copy(
                        out=st["v8"][:, tpi, :, 64:128, :],
                        in_=ps[:].rearrange("p two (h d) -> p h d two", h=NHL),
                    )

            def emit_ctx_ag(st, qj):
                # AllGather the (tiny, fp8) normalized ctx for quarter qj
                # across the quad; each core then runs the FULL Wo matmul
                # locally.  ~8x less CC traffic than AllReducing fp16 deltas.
                l = st["l"]
                swj = slice(qj * QW, (qj + 1) * QW)
                agin = dramp.tile([128, MQ, QW], F8E4, tag="agin",
                                  name=f"agin{l}_{qj}")
                agout = dramp.tile([4, 128, MQ, QW], F8E4, tag="agout",
                                   name=f"agout{l}_{qj}")
                nc.sync.dma_start(agin[:], st["ctxT8"][:, :, swj])
                nc.gpsimd.collective_compute(
                    "AllGather",
                    mybir.AluOpType.bypass,
                    replica_groups=quads,
                    ins=[agin[:].opt()],
                    outs=[agout[:].opt()],
                )
                ctxg = cgp.tile([128, 4, MQ, QW], F8E4, tag="ctxg",
                                name=f"ctxg{l}_{qj}")
                for g in range(4):
                    nc.sync.dma_start(ctxg[:, g, :, :], agout[g, :, :, :])
                st["ctxgs"].append(ctxg)

            def attn_pair(st, qi, m, slots=None):
                """Scores/exp/ctx/normalize for head pair (2m, 2m+1) over
                query quarter qi.  The two heads' K=64 score matmuls run
                concurrently in PE row groups 0-63 / 64-127.  `slots` maps
                tp -> closures emitted at that tp's loop head, used to
                interleave proj / LN PE work into the exp-bound phase."""
                l = st["l"]
                sw = slice(qi * QW, (qi + 1) * QW)
                kp = st["kp2"][m]
                q = st["qT8"][m]
                pctx = [pb.tile([128, QW], F32, tag="pb",
                                name=f"pc{l}_{qi}_{m}_{h}") for h in range(2)]
                probs = [None] * NTP

                def ctx_mm(tp):
                    for h in range(2):
                        nc.tensor.matmul(
                            pctx[h][:],
                            st["v8"][:, tp, 2 * m + h, :, :]
                            .rearrange("p d two -> p (d two)"),
                            probs[tp][:, 2 * h:2 * h + 2, :],
                            start=(tp == 0),
                            stop=(tp == NTP - 1),
                            perf_mode=DRSW,
                        )

                for tp in range(NTP):
                    if slots:
                        for fn in slots.get(tp, []):
                            fn()
                    probs[tp] = prp.tile([128, 4, QW], F8E5, tag="probs",
                                         name=f"pr{l}_{qi}_{m}_{tp}")
                    # probs slots: (h-even t-even, h-even t-odd,
                    #               h-odd t-even, h-odd t-odd)
                    pview = probs[tp][:].rearrange(
                        "p (two pair) n -> p two pair n", two=2)
                    for half in range(2):
                        t = 2 * tp + half
                        ss = ps2.tile([128, 2, QW], F32, tag="ps2")
                        nc.tensor.matmul(ss[:, 0, :], kp[0:64, t, :],
                                         q[0:64, sw], start=True, stop=True)
                        nc.tensor.matmul(ss[:, 1, :], kp[64:128, t, :],
                                         q[64:128, sw], start=True, stop=True)
                        nc.scalar.activation(
                            out=pview[:, :, half, :],
                            in_=ss[:],
                            func=mybir.ActivationFunctionType.Exp,
                            scale=float(1.0 / (np.sqrt(HD) * SW * SW)),
                        )
                    if tp >= LAG:
                        ctx_mm(tp - LAG)
                for tp in range(NTP - LAG, NTP):
                    ctx_mm(tp)

                # normalize both heads: ctx^T * (16 / l[s']), l at psum row
                # 64, ctx dim d at psum row 63-d (SwInterleave reversal).
                # reciprocal_approx_fast misreads PSUM inputs on hw: stage
                # the denominator rows to SBUF first.  (DVE partition bases
                # must be 32-aligned, so rows stay in their own tiles.)
                for h in range(2):
                    lrow = rrp.tile([1, QW], F32, tag="lrow",
                                    name=f"lr_{l}_{qi}_{m}_{h}")
                    nc.vector.tensor_copy(out=lrow[:], in_=pctx[h][64:65, :])
                    r32 = rrp.tile([1, QW], F32, tag="rrow",
                                   name=f"r32_{l}_{qi}_{m}_{h}")
                    nc.vector.reciprocal_approx_fast(out=r32[:], in_=lrow[:])
                    bcs = rrp.tile([64, QW], F32, tag="bcs",
                                   name=f"bcs{l}_{qi}_{m}_{h}")
                    nc.gpsimd.partition_broadcast(bcs[:], r32[:])
                    nc.vector.tensor_mul(
                        out=st["ctxT8"][64 * h:64 * h + 64, m, sw],
                        in0=pctx[h][0:64, :],
                        in1=bcs[:],
                    )

            def ln_wo(st, qi, c0, c1):
                """Full Wo matmul over the gathered quad ctx for feature
                chunks [c0, c1), then out^T = delta/(SW*SW) + (x^T + bo_eff)
                [bo folded host-side] and the square for the LN stats."""
                l = st["l"]
                outT = st["outT"]
                ctxg = st["ctxgs"][qi]
                sw = slice(qi * QW, (qi + 1) * QW)
                if c0 == 0:
                    st.setdefault("sqtq", {})[qi] = sqp.tile(
                        [128, HC, QW], F16, tag="sq", bufs=1,
                        name=f"sqt{l}_{qi}")
                    st["pst"] = pb.tile([128, QW], F32, tag="pb",
                                        name=f"pst{l}_{qi}")
                sqt = st["sqtq"][qi]
                for c in range(c0, c1):
                    pd = pa.tile([128, QW], F32, tag="pa", name=f"pd{l}_{qi}_{c}")
                    for g in range(4):
                        nc.tensor.matmul(
                            pd[:],
                            st["wo"][:, g, c, :],
                            ctxg[:, g, :, :],
                            start=(g == 0),
                            stop=(g == 3),
                            perf_mode=DRSW,
                        )
                    # psum holds 256*delta (16 from ctx scale, 16 from Wo)
                    nc.vector.scalar_tensor_tensor(
                        out=outT[:, c, sw],
                        in0=pd[:],
                        scalar=1.0 / (SW * SW),
                        in1=xT[:, c, sw],
                        op0=mybir.AluOpType.mult,
                        op1=mybir.AluOpType.add,
                    )
                    nc.vector.tensor_mul(
                        out=sqt[:, c, :], in0=outT[:, c, sw], in1=outT[:, c, sw]
                    )
                    # LN stat sums ride right behind each chunk's drain so
                    # the PE never waits on a separate stats tail.
                    nc.tensor.matmul(
                        st["pst"][0:1, :], ones16[:, 0:1], outT[:, c, sw],
                        start=(c == 0), stop=(c == HC - 1),
                        skip_group_check=True,
                    )
                    nc.tensor.matmul(
                        st["pst"][32:33, :], ones16[:, 0:1], sqt[:, c, :],
                        start=(c == 0), stop=(c == HC - 1),
                        skip_group_check=True,
                    )

            def ln_fin(st, qi):
                l, last = st["l"], st["last"]
                outT, lnw_sb = st["outT"], st["lnw"]
                st["sqtq"].pop(qi)
                sw = slice(qi * QW, (qi + 1) * QW)
                pst = st["pst"]
                # LN finalize for this quarter (read sums straight from
                # PSUM: mean = sum/H, var = sumsq/H - mean^2)
                m_sb = lrp.tile([1, QW], F16, tag="lnrow", name=f"m{l}_{qi}")
                nc.vector.tensor_scalar_mul(m_sb[:], pst[0:1, :], 1.0 / H)
                m2 = lrp.tile([1, QW], F16, tag="lnrow", name=f"m2{l}_{qi}")
                nc.vector.tensor_mul(m2[:], m_sb[:], m_sb[:])
                var = lrp.tile([1, QW], F16, tag="lnrow", name=f"va{l}_{qi}")
                nc.vector.scalar_tensor_tensor(
                    out=var[:], in0=pst[32:33, :], scalar=1.0 / H, in1=m2[:],
                    op0=mybir.AluOpType.mult, op1=mybir.AluOpType.subtract,
                )
                # rstd = exp(-0.5 * ln(var + eps)); ln+exp share one ACT table
                lnv = lrp.tile([1, QW], F16, tag="lnrow", name=f"lv{l}_{qi}")
                nc.scalar.activation(
                    out=lnv[:], in_=var[:],
                    func=mybir.ActivationFunctionType.Ln,
                    bias=eps_sb[0:1, :],
                )
                rstd = lrp.tile([1, QW], F16, tag="lnrow", name=f"rs{l}_{qi}")
                nc.scalar.activation(
                    out=rstd[:], in_=lnv[:],
                    func=mybir.ActivationFunctionType.Exp,
                    scale=-0.5,
                )
                # broadcast stats across partitions (gpsimd, off the PE)
                mbs = ltp.tile([128, QW], F16, tag="lntmp", name=f"mbs{l}_{qi}")
                nc.gpsimd.partition_broadcast(mbs[:], m_sb[:])
                rbs = ltp.tile([128, QW], F16, tag="lntmp", name=f"rbs{l}_{qi}")
                nc.gpsimd.partition_broadcast(rbs[:], rstd[:])
                for c in range(HC):
                    tmp = sqp.tile([128, QW], F16, tag="lt", name=f"lt{l}_{qi}_{c}")
                    nc.vector.tensor_sub(out=tmp[:], in0=outT[:, c, sw], in1=mbs[:])
                    nc.vector.scalar_tensor_tensor(
                        out=tmp[:], in0=tmp[:],
                        scalar=lnw_sb[:, c, 0:1], in1=rbs[:],
                        op0=mybir.AluOpType.mult, op1=mybir.AluOpType.mult,
                    )
                    if last:
                        fo = fop.tile([128, QW], F32, tag="fout")
                        nc.vector.tensor_scalar_add(
                            fo[:], tmp[:], lnw_sb[:, c, 1:2]
                        )
                        nc.sync.dma_start(outx[c, :, sw], fo[:])
                    else:
                        nc.vector.tensor_scalar_add(
                            xT8[:, c, sw], tmp[:], lnw_sb[:, c, 1:2]
                        )
                        # xT gets beta + bo_eff of the NEXT layer (slot 2)
                        nc.vector.tensor_scalar_add(
                            xT[:, c, sw], tmp[:], lnw_sb[:, c, 2:3]
                        )

            # ---- schedule: attention's exp chain is the wall; all other PE
            # work (projections of later quarters, the previous quarter's
            # Wo+LN) is sliced into ~1-3us items and slotted between the
            # exp-bound t-pairs so the PE never serializes ahead of the ACT
            # engine.  Quarter qi's scores need proj(qk) of ALL quarters
            # (keys span the sequence): key-quarter X is consumed from tp=2X
            # on, so proj(qX) sits at slot 2X of the very first head pair.
            prev = None
            for l in range(l_layers):
                st = alloc_layer(l)
                proj_qk(st, 0)
                proj_v(st, 0)
                proj_qk(st, 1)
                proj_v(st, 1)
                proj_qk(st, 2)
                proj_v(st, 2)
                if prev is not None:
                    ln_wo(prev, 3, 0, 8)
                    ln_fin(prev, 3)
                proj_qk(st, 3)
                proj_v(st, 3)
                for qi in range(4):
                    attn_pair(st, qi, 0)
                    attn_pair(st, qi, 1)
                    # AG has no PE work -> emit immediately, no head-of-line
                    # blocking of the next quarter's scores
                    emit_ctx_ag(st, qi)
                    if qi >= 2:
                        ln_wo(st, qi - 2, 0, 8)
                        ln_fin(st, qi - 2)
                ln_wo(st, 2, 0, 8)
                ln_fin(st, 2)
                prev = st
            ln_wo(prev, 3, 0, 8)
            ln_fin(prev, 3)
    nc.compile()
    return nc


def make_in_maps(inputs, s=S, l_layers=L):
    """Host-side sharding: returns one input dict per core."""
    x = np.asarray(inputs["input_tensor"], dtype=np.float32)      # [s, B, H]
    Wq = np.asarray(inputs["Wq"], dtype=np.float32)[:l_layers]
    Wk = np.asarray(inputs["Wk"], dtype=np.float32)[:l_layers]
    Wv = np.asarray(inputs["Wv"], dtype=np.float32)[:l_layers]
    Wo = np.asarray(inputs["Wo"], dtype=np.float32)[:l_layers]
    bq = np.asarray(inputs["bq"], dtype=np.float32)[:l_layers]
    bk = np.asarray(inputs["bk"], dtype=np.float32)[:l_layers]
    bv = np.asarray(inputs["bv"], dtype=np.float32)[:l_layers]
    bo = np.asarray(inputs["bo"], dtype=np.float32)[:l_layers]
    gamma = np.asarray(inputs["gamma"], dtype=np.float32)[:l_layers]
    beta = np.asarray(inputs["beta"], dtype=np.float32)[:l_layers]
    ll = l_layers

    # bv passes through the softmax-weighted sum exactly: fold bv@Wo into bo.
    bo_eff = bo + np.einsum("lh,lhk->lk", bv, Wo)
    # xT carries x + bo_eff of the upcoming layer; lnw slot 2 = beta +
    # bo_eff(next layer) so the residual add is a pure tensor_tensor op.
    sl2 = beta.copy()
    sl2[:ll - 1] += bo_eff[1:]

    def chunkP(a, n_out):
        # [..., n_out*128, inner] -> [..., 128, n_out, inner] feature-chunked
        sh = a.shape
        a = a.reshape(*sh[:-2], n_out, 128, sh[-1])
        return np.moveaxis(a, -3, -2)  # -> [..., 128, n_out, inner]

    e4 = ml_dtypes.float8_e4m3

    def sw_interleave(A, Bm):
        # A, Bm: [..., K, M] -> [..., K, 2M] canonical SwInterleave layout:
        # stored cols [A(M-1) B(M-1) ... A(0) B(0)]
        st = np.stack([A[..., ::-1], Bm[..., ::-1]], axis=-1)
        return st.reshape(*st.shape[:-2], -1)

    def qk_prep(W):
        # [L,H,DQ]*SW -> [L, 128, HC//2, MQ, 256] SwInterleave over c-pairs
        Wc = (W * SW).reshape(ll, HC, 128, DQ)       # [L, c, p, DQ]
        out = np.empty((ll, 128, HC // 2, MQ, 256), np.float32)
        for c2 in range(HC // 2):
            for m in range(MQ):
                A = Wc[:, 2 * c2, :, m * 128:(m + 1) * 128]
                Bm = Wc[:, 2 * c2 + 1, :, m * 128:(m + 1) * 128]
                out[:, :, c2, m, :] = sw_interleave(A, Bm)
        return out

    # ctxT8 partition p (within m-chunk par) holds head 2*par + (p>=64),
    # dim d = 63 - (p % 64); permute Wo rows to match before interleaving.
    k_idx = np.arange(128)

    def wo_prep(Wc):
        # Wc: [L, DQ, H]*SW -> [L, 128, HC, 256] (rows permuted + interleaved)
        Wp = np.empty((ll, 2, 128, H), np.float32)
        for par in range(2):
            f = 64 * (2 * par + (k_idx >= 64)) + (63 - (k_idx % 64))
            Wp[:, par, :, :] = Wc[:, f, :] * SW
        out = np.empty((ll, 128, HC, 256), np.float32)
        for c in range(HC):
            out[:, :, c, :] = sw_interleave(
                Wp[:, 0, :, c * 128:(c + 1) * 128],
                Wp[:, 1, :, c * 128:(c + 1) * 128],
            )
        return out

    # Full Wo per core (AllGather-ctx): one permuted+interleaved slice per
    # quad member g; identical on every core.
    wo = np.ascontiguousarray(np.stack(
        [wo_prep(Wo[:, DQ * g:DQ * (g + 1), :]) for g in range(4)], axis=2
    ))

    in_maps = []
    for core in range(N_CORES):
        g, j = core // 4, core % 4
        cols = slice(DQ * j, DQ * (j + 1))
        xT = np.ascontiguousarray(x[:, g, :].T).reshape(HC, 128, s)
        xTb = xT + bo_eff[0].reshape(HC, 128, 1)
        wq = np.ascontiguousarray(qk_prep(Wq[:, :, cols]))
        wk = np.ascontiguousarray(qk_prep(Wk[:, :, cols]))
        wv = np.ascontiguousarray(chunkP(Wv[:, :, cols] * SW, HC))
        bqs = bq[:, cols].reshape(ll, MQ, 128).transpose(0, 2, 1)  # [L,128,MQ]
        bks = bk[:, cols].reshape(ll, MQ, 128).transpose(0, 2, 1)
        bqk = np.ascontiguousarray(np.concatenate([bqs, bks], axis=2)) * SW
        lnw = np.stack(
            [
                gamma.reshape(ll, HC, 128).transpose(0, 2, 1),
                beta.reshape(ll, HC, 128).transpose(0, 2, 1),
                sl2.reshape(ll, HC, 128).transpose(0, 2, 1),
            ],
            axis=3,
        )                                                          # [L,128,HC,3]
        in_maps.append(
            {
                "xT0": xTb.astype(np.float16),
                "xT80": xT.astype(e4),
                "wq": wq.astype(e4),
                "wk": wk.astype(e4),
                "wv": wv.astype(e4),
                "wo": wo.astype(e4),
                "bqk": bqk.astype(np.float32),
                "lnw": np.ascontiguousarray(lnw).astype(np.float32),
            }
        )
    return in_maps


_NC_CACHE = {}


def kernel(**inputs) -> np.ndarray:
    in_maps = make_in_maps(inputs)
    key = (S, L)
    if key not in _NC_CACHE:
        _NC_CACHE[key] = build_bass()
    nc = _NC_CACHE[key]
    res = run_bass_kernel_spmd(nc, in_maps, core_ids=list(range(N_CORES)))
    out = np.empty((S, B, H), dtype=np.float32)
    for g, core in ((0, 0), (1, 4)):
        xt = res.results[core]["outx"].reshape(H, S)
        out[:, g, :] = xt.T
    return out
